# revision 25
# baseline (speedup 1.0000x reference)
"""MultiHeadCrossAttentionFusion kernel for TRN2 (8 NeuronCores, data-parallel over batch).

Per-core design (batch shard BS=1024, processed in 2 chunks of 512 rows):
  - QKV matmuls computed directly in TRANSPOSED layout (weights stationary,
    xT streaming) so attention reads q/k/v with the head dim on partitions.
  - Attention packs 8 samples x 16 heads on partitions; the block-diagonal
    softmax mask rides the score matmul as 9 extra contraction rows
    (mask = 800*delta_bb' - 800 expressed as rank-9 outer products).
  - Softmax denominator via a 65th ones-column appended to V.
  - LN stats via ones-matmuls; all sqrt ops batched (no exp/sqrt ACT-table
    thrash); 1/sd applied to caT with a broadcast-AP multiply; mu*un + v
    folded into the projection as a single K=2 matmul.
  - Projection from caT (stationary) streaming LN-folded weights; residual
    added from a fresh x DMA.
"""
import sys
sys.path.insert(0, "/opt/trn_rl_repo")
import numpy as np
import ml_dtypes
from contextlib import ExitStack

import concourse.bass as bass
from concourse import bacc as _bacc
import concourse.mybir as mybir
from concourse.tile import TileContext
from concourse.bass_utils import run_bass_kernel_spmd

B, CD, HID, H, D = 8192, 2048, 1024, 16, 64
NCORES = 8
BS = B // NCORES          # 1024 rows per core
CG = 512                  # chunk rows
NCHUNK = BS // CG         # 2
KT = CD // 128            # 16 k-tiles for qkv matmul
NT = HID // 128           # 8 n-tiles per role (q/k/v)
CT = HID // 128           # 8 c-tiles for proj contraction
NCH_P = CD // 512         # 4 n-chunks of proj
EPS = 1e-5
MS = 800.0                # mask magnitude (scaled by 1/8 in exp -> -100)
F32 = mybir.dt.float32
BF16 = mybir.dt.bfloat16
AL = mybir.AluOpType
AF = mybir.ActivationFunctionType


def _bc_ap(row_ap, p, reps, n):
    """Broadcast a [1, n] row AP to [p, reps, n] via zero strides."""
    return bass.AP(tensor=row_ap.tensor, offset=row_ap.offset,
                   ap=[[0, p], [0, reps], list(row_ap.ap)[-1][:]])


def build_nc(linearize=False):
    nc = _bacc.Bacc()
    dp = nc.declare_dram_parameter
    xT_d = {"c": dp("xT_c", [CD, BS], BF16, isOutput=False),
            "m": dp("xT_m", [CD, BS], BF16, isOutput=False)}
    Wq = {"c": dp("Wq_c", [CD, 3 * HID], BF16, isOutput=False),
          "m": dp("Wq_m", [CD, 3 * HID], BF16, isOutput=False)}
    bqT = {"c": dp("bqT_c", [128, 3 * NT], F32, isOutput=False),
           "m": dp("bqT_m", [128, 3 * NT], F32, isOutput=False)}
    Wg = {"c": dp("Wg_c", [HID, CD], BF16, isOutput=False),
          "m": dp("Wg_m", [HID, CD], BF16, isOutput=False)}
    xv = {"c": dp("xv_c", [BS, CD], BF16, isOutput=False),
          "m": dp("xv_m", [BS, CD], BF16, isOutput=False)}
    un_d = dp("un_all", [1, 2 * CD], BF16, isOutput=False)
    kext_d = dp("kext", [9, 2048], BF16, isOutput=False)
    qext_d = dp("qext", [9, 2048], BF16, isOutput=False)
    identb = dp("identb", [128, 128], BF16, isOutput=False)
    ones_col_d = dp("ones_col", [128, 1], BF16, isOutput=False)
    onesr_d = dp("onesr", [1, 128], BF16, isOutput=False)
    out = {"c": dp("out_c", [BS, CD], F32, isOutput=True),
           "m": dp("out_m", [BS, CD], F32, isOutput=True)}

    with TileContext(nc, linearize=linearize) as tc, ExitStack() as ctx:
        consts = ctx.enter_context(tc.tile_pool(name="consts", bufs=1))
        keep = ctx.enter_context(tc.tile_pool(name="keep", bufs=1))
        psQ = ctx.enter_context(tc.tile_pool(name="psQ", bufs=2, space="PSUM"))
        psT = ctx.enter_context(tc.tile_pool(name="psT", bufs=2, space="PSUM"))
        psS = ctx.enter_context(tc.tile_pool(name="psS", bufs=2, space="PSUM"))
        psCU = ctx.enter_context(tc.tile_pool(name="psCU", bufs=2, space="PSUM"))
        wst_p = ctx.enter_context(tc.tile_pool(name="wstp", bufs=2))
        apool = ctx.enter_context(tc.tile_pool(name="apool", bufs=2))
        stp = ctx.enter_context(tc.tile_pool(name="stp", bufs=4))
        wgp = ctx.enter_context(tc.tile_pool(name="wgp", bufs=2))
        tmpC = ctx.enter_context(tc.tile_pool(name="tmpC", bufs=2))

        # ---- constants
        sb_id = consts.tile([128, 128], BF16)
        nc.sync.dma_start(sb_id, identb[:, :])
        ones_col = consts.tile([128, 1], BF16)
        nc.sync.dma_start(ones_col, ones_col_d[:, :])
        _ = onesr_d  # unused (kept as a declared param for layout stability)
        sb_un_all = consts.tile([1, 2 * CD], BF16, tag="un")
        nc.sync.dma_start(sb_un_all, un_d[:, :])
        sb_un = {"c": sb_un_all[:, 0:CD], "m": sb_un_all[:, CD:2 * CD]}
        sb_bqT = {}
        for t in ("c", "m"):
            sb_bqT[t] = consts.tile([128, 3 * NT], F32, name=f"bqT_{t}",
                                    tag=f"bqT_{t}")
            nc.sync.dma_start(sb_bqT[t], bqT[t][:, :])

        # ---- persistent cross-chunk tiles
        caT_all = keep.tile([128, 16 * (H // 2), 128], BF16, tag="caT_all")
        rcol = keep.tile([128, 16], F32, tag="rcol")

        # per-chunk qkv-transposed + xT tiles (bufs=1 -> reused across chunks)
        xT = {t: keep.tile([128, KT, CG], BF16, name=f"xT_{t}", tag=f"xT_{t}")
              for t in ("c", "m")}
        qkvT = {}
        for t in ("c", "m"):
            for role in ("q", "k", "v"):
                qkvT[(role, t)] = keep.tile(
                    [128, NT, CG], BF16, name=f"{role}T_{t}",
                    tag=f"{role}T_{t}")

        def qkv_group(role, t, nt):
            """One n-tile of QKV for role/branch on the current chunk rows."""
            roff = {"q": 0, "k": NT, "v": 2 * NT}[role]
            px = psQ.tile([128, CG], F32, tag="px")
            for kh in range(2):
                wst = wst_p.tile([128, KT // 2, 128], BF16, tag="wst",
                                 bufs=3)
                nc.sync.dma_start(
                    wst,
                    Wq[t][kh * 1024:(kh + 1) * 1024,
                          (roff + nt) * 128:(roff + nt + 1) * 128]
                    .rearrange("(kt p) n -> p kt n", p=128))
                for kk in range(KT // 2):
                    kt = kh * 8 + kk
                    nc.tensor.matmul(px, lhsT=wst[:, kk, :],
                                     rhs=xT[t][:, kt, :],
                                     start=(kt == 0),
                                     stop=(kt == KT - 1))
            nc.scalar.activation(
                qkvT[(role, t)][:, nt, :], px, AF.Identity,
                bias=sb_bqT[t][:, roff + nt:roff + nt + 1])

        def attn_packs(bl, qt, kt_b):
            """Build packed K/Q/V tiles for one b-tile (DVE + tiny DMAs)."""
            rows = slice(bl * 128, (bl + 1) * 128)
            kpk = apool.tile([128, 2048], BF16, tag="kpk")
            qpk = apool.tile([128, 2048], BF16, tag="qpk")
            vpk = apool.tile([128, 2048], BF16, tag="vpk")
            nc.sync.dma_start(kpk[64:73, :], kext_d[:, :])
            nc.sync.dma_start(qpk[64:73, :], qext_d[:, :])
            for par in range(2):
                for src_t, dst in (((("k", kt_b)), kpk), ((("q", qt)), qpk),
                                   ((("v", kt_b)), vpk)):
                    s = qkvT[src_t][par * 64:(par + 1) * 64, :, rows]\
                        .rearrange("d ge (j b) -> d j ge b", b=8)
                    o = dst[0:64, :]\
                        .rearrange("d (j ge pp b) -> d j ge pp b",
                                   j=16, ge=8, pp=2)[:, :, :, par, :]
                    nc.vector.tensor_copy(out=o, in_=s)
            return kpk, qpk, vpk

        def attn_compute(u, bl, packs, mu_h, vv_h):
            """Attention math for one b-tile from its packed tiles."""
            kpk, qpk, vpk = packs
            # scores + exp, 4 j-groups per PSUM bank
            eT = apool.tile([128, 2048], BF16, tag="eT")
            for sb in range(4):
                sp = psS.tile([128, 512], F32, tag="sp")
                for q in range(4):
                    j = sb * 4 + q
                    nc.tensor.matmul(
                        sp[:, q * 128:(q + 1) * 128],
                        lhsT=kpk[0:73, j * 128:(j + 1) * 128],
                        rhs=qpk[0:73, j * 128:(j + 1) * 128],
                        start=True, stop=True)
                nc.scalar.activation(eT[:, sb * 512:(sb + 1) * 512], sp,
                                     AF.Exp, scale=0.125)
            # vp = transpose(vpk)
            vp = apool.tile([128, 16 * 64], BF16, tag="vp")
            for jb in range(2):
                vt = psT.tile([128, 8 * 64], BF16, tag="pt")
                for jj in range(8):
                    j = jb * 8 + jj
                    nc.tensor.transpose(
                        vt[:, jj * 64:(jj + 1) * 64],
                        vpk[0:64, j * 128:(j + 1) * 128],
                        sb_id[0:64, 0:64])
                nc.vector.tensor_copy(
                    out=vp[:, jb * 512:(jb + 1) * 512], in_=vt)
            # weighted sums + batched row-sum reciprocals + normalize
            caU = apool.tile([128, 1024], BF16, tag="caU", bufs=1)
            for jb in range(2):
                cua = psCU.tile([128, 512], F32, tag="cu")
                cus = psS.tile([128, 8], F32, tag="sp")
                for jj in range(8):
                    j = jb * 8 + jj
                    nc.tensor.matmul(cua[:, jj * 64:(jj + 1) * 64],
                                     lhsT=eT[:, j * 128:(j + 1) * 128],
                                     rhs=vp[:, j * 64:(j + 1) * 64],
                                     start=True, stop=True)
                    nc.tensor.matmul(cus[:, jj:jj + 1],
                                     lhsT=eT[:, j * 128:(j + 1) * 128],
                                     rhs=ones_col,
                                     start=True, stop=True)
                rcz = stp.tile([128, 8], F32, tag="rcz")
                nc.vector.reciprocal(rcz, cus)
                for jj in range(8):
                    j = jb * 8 + jj
                    nc.vector.tensor_scalar(
                        out=caU[:, j * 64:(j + 1) * 64],
                        in0=cua[:, jj * 64:(jj + 1) * 64],
                        scalar1=rcz[:, jj:jj + 1], scalar2=None,
                        op0=AL.mult)
            # transpose caU -> caT_all[:, u*8:(u+1)*8, :]
            for jb in range(2):
                ct = psT.tile([64, 8, 128], BF16, tag="pt")
                for jj in range(8):
                    j = jb * 8 + jj
                    nc.tensor.transpose(ct[0:64, jj, :],
                                        caU[:, j * 64:(j + 1) * 64], sb_id)
                for par in range(2):
                    s = ct[0:64, :, :].rearrange(
                        "d j (hp pp b) -> d hp pp j b", pp=2, b=8)[:, :, par]
                    o = caT_all[par * 64:(par + 1) * 64,
                                u * 8:(u + 1) * 8,
                                jb * 64:(jb + 1) * 64]\
                        .rearrange("d hp (j b) -> d hp j b", b=8)
                    nc.scalar.copy(out=o, in_=s)
            # LN stats (no sqrt here -- batched later)
            sq = apool.tile([128, H // 2, 128], BF16, tag="sq", bufs=1)
            cslice = caT_all[:, u * 8:(u + 1) * 8, :]
            nc.vector.tensor_tensor(out=sq, in0=cslice, in1=cslice,
                                    op=AL.mult)
            mrow = psS.tile([1, 128], F32, tag="sp")
            srow = psS.tile([1, 128], F32, tag="sp")
            for hp in range(H // 2):
                nc.tensor.matmul(mrow, lhsT=ones_col,
                                 rhs=caT_all[:, u * 8 + hp, :],
                                 start=(hp == 0), stop=(hp == 7))
                nc.tensor.matmul(srow, lhsT=ones_col, rhs=sq[:, hp, :],
                                 start=(hp == 0), stop=(hp == 7))
            mus = mu_h[:, bl * 128:(bl + 1) * 128]
            nc.vector.tensor_scalar(
                out=mus, in0=mrow,
                scalar1=1.0 / HID, scalar2=None, op0=AL.mult)
            s2 = stp.tile([1, 128], F32, tag="s2", bufs=2)
            nc.vector.tensor_scalar(out=s2, in0=srow, scalar1=1.0 / HID,
                                    scalar2=EPS, op0=AL.mult, op1=AL.add)
            mu2 = stp.tile([1, 128], F32, tag="mu2", bufs=2)
            nc.vector.tensor_tensor(out=mu2, in0=mus, in1=mus, op=AL.mult)
            nc.vector.tensor_tensor(out=vv_h[:, bl * 128:(bl + 1) * 128],
                                    in0=s2, in1=mu2, op=AL.subtract)

        def proj_half(u0, chunk, t, mu_h, vv_h, extra=()):
            """Projection + residual for units u0..u0+3 (branch t).
            extra: up to 16 thunks interleaved across the 4 n-chunks."""
            # r = 1/sqrt(vv) -> per-sample columns of rcol for the ACT scale
            nc.scalar.activation(vv_h, vv_h, AF.Sqrt)
            rb = stp.tile([1, 512], BF16, tag="rb", bufs=1)
            with nc.allow_low_precision(reason="1/sd as bf16 scale factor"):
                nc.vector.reciprocal(rb, vv_h)
            rcp = psCU.tile([128, 8], BF16, tag="cu")
            for bl in range(CG // 128):
                nc.tensor.transpose(rcp[:, 2 * bl:2 * bl + 1],
                                    rb[:, bl * 128:(bl + 1) * 128],
                                    sb_id[0:1, 0:1])
            nc.vector.tensor_copy(
                out=rcol[:, u0:u0 + 4],
                in_=rcp.rearrange("p (f two) -> p f two", two=2)[:, :, 0])
            for nch in range(NCH_P):
                for fn in extra[nch * 4:(nch + 1) * 4]:
                    fn()
                wg = wgp.tile([128, CT, 512], BF16, tag="wg")
                nc.sync.dma_start(
                    wg, Wg[t][:, nch * 512:(nch + 1) * 512]
                    .rearrange("(ct p) n -> p ct n", p=128))
                for bl in range(CG // 128):
                    u = u0 + bl
                    rows = slice(chunk * CG + bl * 128,
                                 chunk * CG + (bl + 1) * 128)
                    cslice = caT_all[:, u * 8:(u + 1) * 8, :]
                    px = psQ.tile([128, 512], F32, tag="px")
                    for ct in range(CT):
                        nc.tensor.matmul(px, lhsT=cslice[:, ct, :],
                                         rhs=wg[:, ct, :],
                                         start=(ct == 0), stop=False)
                    nc.tensor.matmul(
                        px, lhsT=mu_h[:, bl * 128:(bl + 1) * 128],
                        rhs=sb_un[t][:, nch * 512:(nch + 1) * 512],
                        start=False, stop=True)
                    nc.scalar.activation(px, px, AF.Copy,
                                         scale=rcol[:, u:u + 1])
                    xres = tmpC.tile([128, 512], BF16, tag="xres")
                    nc.sync.dma_start(
                        xres, xv[t][rows, nch * 512:(nch + 1) * 512])
                    ot = tmpC.tile([128, 512], F32, tag="ot")
                    nc.vector.tensor_tensor(out=ot, in0=px, in1=xres,
                                            op=AL.add)
                    nc.sync.dma_start(
                        out[t][rows, nch * 512:(nch + 1) * 512], ot)

        def xT_load(chunk, t):
            nc.sync.dma_start(
                xT[t],
                xT_d[t][:, chunk * CG:(chunk + 1) * CG]
                .rearrange("(kt p) r -> p kt r", p=128))

        # ================= main schedule =================
        # Flat 4-half software pipeline over halves i = chunk*2 + half:
        #   qkv(0) | attn(0) x qkv(1) | proj(0) x xT(chunk2) |
        #   attn(1) x qkv(2) | proj(1) | attn(2) x qkv(3) | proj(2) |
        #   attn(3) | proj(3)
        # so the PE always has dense matmul work while attention's
        # DVE/ACT latency chains run underneath.
        def half_params(i):
            chunk, half = divmod(i, 2)
            qt = "c" if half == 0 else "m"
            kt_b = "m" if half == 0 else "c"
            return chunk, half, qt, kt_b

        def qkv_pairs(i):
            _, _, qt, kt_b = half_params(i)
            return [(role, tt, nt)
                    for role, tt in (("q", qt), ("k", kt_b), ("v", kt_b))
                    for nt in range(NT)]

        for t in ("c", "m"):
            xT_load(0, t)
        for pr in qkv_pairs(0):
            qkv_group(*pr)
        for i in range(4):
            chunk, half, qt, kt_b = half_params(i)
            u0 = i * 4
            mu_h = stp.tile([1, 512], BF16, tag="mu_h", bufs=2)
            vv_h = stp.tile([1, 512], F32, tag="vv_h", bufs=2)
            nxt = qkv_pairs(i + 1) if i < 3 else []
            # emit packs one filler-window ahead of their compute so the
            # tail unit's inputs are ready when the PE reaches it
            packs = {0: attn_packs(0, qt, kt_b)}
            for bl in range(CG // 128):
                for pr in nxt[bl * 6:(bl + 1) * 6]:
                    qkv_group(*pr)
                if bl + 1 < CG // 128:
                    packs[bl + 1] = attn_packs(bl + 1, qt, kt_b)
                attn_compute(u0 + bl, bl, packs.pop(bl), mu_h, vv_h)
            extra = []
            if i == 0:
                extra = [(lambda tt=t2: xT_load(1, tt))
                         for t2 in ("c", "m")]
            proj_half(u0, chunk, qt, mu_h, vv_h, extra=extra)
    return nc


_NC = {}


def _get_nc():
    if "nc" not in _NC:
        nc = build_nc()
        if not nc.is_finalized():
            nc.finalize()
        _NC["nc"] = nc
    return _NC["nc"]


def _host_prep(inputs):
    f32 = np.float32
    bf = ml_dtypes.bfloat16
    g = {k: np.asarray(v) for k, v in inputs.items()}
    # permutation: device caT row c_dev (hp*128 + p) <-> ref column d*16+h
    cdev = np.arange(HID)
    hp_t, p_t = cdev // 128, cdev % 128
    h_t = 2 * hp_t + (p_t // 64)
    d_t = p_t % 64
    pr = d_t * H + h_t                   # ref row for each device row
    consts = {}
    for t, (Wp, bp, g1, be1) in (
            ("c", ("W_cproj", "b_cproj", "g1", "be1")),
            ("m", ("W_mproj", "b_mproj", "g2", "be2"))):
        W = np.asarray(g[Wp], f32)[pr, :]          # [HID, CD] permuted
        g1d = np.asarray(g[g1], f32)[pr]
        be1d = np.asarray(g[be1], f32)[pr]
        consts[f"Wg_{t}"] = np.ascontiguousarray(
            (g1d[:, None] * W)).astype(bf)
        consts[f"un_{t}"] = (-(g1d[:, None] * W).sum(0)).reshape(1, CD)
        consts[f"v_{t}"] = (be1d @ W + np.asarray(g[bp], f32)).reshape(1, CD)
    consts["un_all"] = np.concatenate(
        [consts.pop("un_c"), consts.pop("un_m")], 1).astype(bf)
    consts["Wq_c"] = np.asarray(g["W_cqkv"], f32).astype(bf)
    consts["Wq_m"] = np.asarray(g["W_mqkv"], f32).astype(bf)
    consts["bqT_c"] = np.ascontiguousarray(
        np.asarray(g["b_cqkv"], f32).reshape(3 * NT, 128).T)
    consts["bqT_m"] = np.ascontiguousarray(
        np.asarray(g["b_mqkv"], f32).reshape(3 * NT, 128).T)
    # mask extension rows: sum_i kext[i,(g,b)]*qext[i,(h,b')] = MS*(b==b') - MS
    col_b = np.tile(np.arange(128) % 8, 16)        # b index per packed column
    kext = np.zeros((9, 2048), f32)
    qext = np.zeros((9, 2048), f32)
    for i in range(8):
        kext[i] = np.where(col_b == i, MS, 0.0)
        qext[i] = np.where(col_b == i, 1.0, 0.0)
    kext[8] = -MS
    qext[8] = 1.0
    consts["kext"] = kext.astype(bf)
    consts["qext"] = qext.astype(bf)
    consts["identb"] = np.eye(128).astype(bf)
    consts["ones_col"] = np.ones((128, 1)).astype(bf)
    consts["onesr"] = np.ones((1, 128)).astype(bf)
    return g, consts


def kernel(**inputs):
    g, consts = _host_prep(inputs)
    xc = np.ascontiguousarray(np.asarray(g["cnn_out"], np.float32))
    xm = np.ascontiguousarray(np.asarray(g["mlp_out"], np.float32))
    nc = _get_nc()
    v_c = consts.pop("v_c").astype(np.float32)
    v_m = consts.pop("v_m").astype(np.float32)
    bf = ml_dtypes.bfloat16
    xvc = (xc + v_c).astype(bf)
    xvm = (xm + v_m).astype(bf)
    xcb = xc.astype(bf)
    xmb = xm.astype(bf)
    in_maps = []
    for i in range(NCORES):
        m = dict(consts)
        m["xT_c"] = np.ascontiguousarray(xcb[i * BS:(i + 1) * BS].T)
        m["xT_m"] = np.ascontiguousarray(xmb[i * BS:(i + 1) * BS].T)
        m["xv_c"] = xvc[i * BS:(i + 1) * BS]
        m["xv_m"] = xvm[i * BS:(i + 1) * BS]
        in_maps.append(m)
    res = run_bass_kernel_spmd(nc, in_maps, list(range(NCORES))).results
    out_c = np.concatenate([np.asarray(res[i]["out_c"]) for i in range(NCORES)], 0)
    out_m = np.concatenate([np.asarray(res[i]["out_m"]) for i in range(NCORES)], 0)
    return (out_c.astype(np.float32), out_m.astype(np.float32))


# revision 27
# speedup vs baseline: 1.0118x; 1.0118x over previous
"""MultiHeadCrossAttentionFusion kernel for TRN2 (8 NeuronCores, data-parallel over batch).

Per-core design (batch shard BS=1024, processed in 2 chunks of 512 rows):
  - QKV matmuls computed directly in TRANSPOSED layout (weights stationary,
    xT streaming) so attention reads q/k/v with the head dim on partitions.
  - Attention packs 8 samples x 16 heads on partitions; the block-diagonal
    softmax mask rides the score matmul as 9 extra contraction rows
    (mask = 800*delta_bb' - 800 expressed as rank-9 outer products).
  - Softmax denominator via a 65th ones-column appended to V.
  - LN stats via ones-matmuls; all sqrt ops batched (no exp/sqrt ACT-table
    thrash); 1/sd applied to caT with a broadcast-AP multiply; mu*un + v
    folded into the projection as a single K=2 matmul.
  - Projection from caT (stationary) streaming LN-folded weights; residual
    added from a fresh x DMA.
"""
import sys
sys.path.insert(0, "/opt/trn_rl_repo")
import numpy as np
import ml_dtypes
from contextlib import ExitStack

import concourse.bass as bass
from concourse import bacc as _bacc
import concourse.mybir as mybir
from concourse.tile import TileContext
from concourse.bass_utils import run_bass_kernel_spmd

B, CD, HID, H, D = 8192, 2048, 1024, 16, 64
NCORES = 8
BS = B // NCORES          # 1024 rows per core
CG = 512                  # chunk rows
NCHUNK = BS // CG         # 2
KT = CD // 128            # 16 k-tiles for qkv matmul
NT = HID // 128           # 8 n-tiles per role (q/k/v)
CT = HID // 128           # 8 c-tiles for proj contraction
NCH_P = CD // 512         # 4 n-chunks of proj
EPS = 1e-5
MS = 800.0                # mask magnitude (scaled by 1/8 in exp -> -100)
F32 = mybir.dt.float32
BF16 = mybir.dt.bfloat16
AL = mybir.AluOpType
AF = mybir.ActivationFunctionType


def _bc_ap(row_ap, p, reps, n):
    """Broadcast a [1, n] row AP to [p, reps, n] via zero strides."""
    return bass.AP(tensor=row_ap.tensor, offset=row_ap.offset,
                   ap=[[0, p], [0, reps], list(row_ap.ap)[-1][:]])


def build_nc(linearize=False):
    nc = _bacc.Bacc()
    dp = nc.declare_dram_parameter
    xT_d = {"c": dp("xT_c", [CD, BS], BF16, isOutput=False),
            "m": dp("xT_m", [CD, BS], BF16, isOutput=False)}
    Wq = {"c": dp("Wq_c", [CD, 3 * HID], BF16, isOutput=False),
          "m": dp("Wq_m", [CD, 3 * HID], BF16, isOutput=False)}
    bqT = {"c": dp("bqT_c", [128, 3 * NT], F32, isOutput=False),
           "m": dp("bqT_m", [128, 3 * NT], F32, isOutput=False)}
    Wg = {"c": dp("Wg_c", [HID, CD], BF16, isOutput=False),
          "m": dp("Wg_m", [HID, CD], BF16, isOutput=False)}
    xv = {"c": dp("xv_c", [BS, CD], BF16, isOutput=False),
          "m": dp("xv_m", [BS, CD], BF16, isOutput=False)}
    un_d = dp("un_all", [1, 2 * CD], BF16, isOutput=False)
    kext_d = dp("kext", [9, 2048], BF16, isOutput=False)
    qext_d = dp("qext", [9, 2048], BF16, isOutput=False)
    identb = dp("identb", [128, 128], BF16, isOutput=False)
    ones_col_d = dp("ones_col", [128, 1], BF16, isOutput=False)
    onesr_d = dp("onesr", [1, 128], BF16, isOutput=False)
    out = {"c": dp("out_c", [BS, CD], F32, isOutput=True),
           "m": dp("out_m", [BS, CD], F32, isOutput=True)}

    with TileContext(nc, linearize=linearize) as tc, ExitStack() as ctx:
        consts = ctx.enter_context(tc.tile_pool(name="consts", bufs=1))
        keep = ctx.enter_context(tc.tile_pool(name="keep", bufs=1))
        psQ = ctx.enter_context(tc.tile_pool(name="psQ", bufs=2, space="PSUM"))
        psT = ctx.enter_context(tc.tile_pool(name="psT", bufs=2, space="PSUM"))
        psS = ctx.enter_context(tc.tile_pool(name="psS", bufs=2, space="PSUM"))
        psCU = ctx.enter_context(tc.tile_pool(name="psCU", bufs=2, space="PSUM"))
        wst_p = ctx.enter_context(tc.tile_pool(name="wstp", bufs=2))
        apool = ctx.enter_context(tc.tile_pool(name="apool", bufs=2))
        stp = ctx.enter_context(tc.tile_pool(name="stp", bufs=4))
        wgp = ctx.enter_context(tc.tile_pool(name="wgp", bufs=2))
        tmpC = ctx.enter_context(tc.tile_pool(name="tmpC", bufs=2))

        # ---- constants
        sb_id = consts.tile([128, 128], BF16)
        nc.sync.dma_start(sb_id, identb[:, :])
        ones_col = consts.tile([128, 1], BF16)
        nc.sync.dma_start(ones_col, ones_col_d[:, :])
        _ = onesr_d  # unused (kept as a declared param for layout stability)
        sb_un_all = consts.tile([1, 2 * CD], BF16, tag="un")
        nc.sync.dma_start(sb_un_all, un_d[:, :])
        sb_un = {"c": sb_un_all[:, 0:CD], "m": sb_un_all[:, CD:2 * CD]}
        sb_bqT = {}
        for t in ("c", "m"):
            sb_bqT[t] = consts.tile([128, 3 * NT], F32, name=f"bqT_{t}",
                                    tag=f"bqT_{t}")
            nc.sync.dma_start(sb_bqT[t], bqT[t][:, :])

        # ---- persistent cross-chunk tiles
        caT_all = keep.tile([128, 16 * (H // 2), 128], BF16, tag="caT_all")
        rcol = keep.tile([128, 16], F32, tag="rcol")

        # per-chunk qkv-transposed + xT tiles (bufs=1 -> reused across chunks)
        xT = {t: keep.tile([128, KT, CG], BF16, name=f"xT_{t}", tag=f"xT_{t}")
              for t in ("c", "m")}
        qkvT = {}
        for t in ("c", "m"):
            for role in ("q", "k", "v"):
                qkvT[(role, t)] = keep.tile(
                    [128, NT, CG], BF16, name=f"{role}T_{t}",
                    tag=f"{role}T_{t}")

        def qkv_group(role, t, nt):
            """One n-tile of QKV for role/branch on the current chunk rows."""
            roff = {"q": 0, "k": NT, "v": 2 * NT}[role]
            px = psQ.tile([128, CG], F32, tag="px")
            for kh in range(2):
                wst = wst_p.tile([128, KT // 2, 128], BF16, tag="wst",
                                 bufs=3)
                nc.sync.dma_start(
                    wst,
                    Wq[t][kh * 1024:(kh + 1) * 1024,
                          (roff + nt) * 128:(roff + nt + 1) * 128]
                    .rearrange("(kt p) n -> p kt n", p=128))
                for kk in range(KT // 2):
                    kt = kh * 8 + kk
                    nc.tensor.matmul(px, lhsT=wst[:, kk, :],
                                     rhs=xT[t][:, kt, :],
                                     start=(kt == 0),
                                     stop=(kt == KT - 1))
            nc.scalar.activation(
                qkvT[(role, t)][:, nt, :], px, AF.Identity,
                bias=sb_bqT[t][:, roff + nt:roff + nt + 1])

        def attn_packs(bl, qt, kt_b):
            """Build packed K/Q/V tiles for one b-tile (DVE + tiny DMAs)."""
            rows = slice(bl * 128, (bl + 1) * 128)
            kpk = apool.tile([128, 2048], BF16, tag="kpk")
            qpk = apool.tile([128, 2048], BF16, tag="qpk")
            vpk = apool.tile([128, 2048], BF16, tag="vpk")
            nc.sync.dma_start(kpk[64:73, :], kext_d[:, :])
            nc.sync.dma_start(qpk[64:73, :], qext_d[:, :])
            for par in range(2):
                for src_t, dst in (((("k", kt_b)), kpk), ((("q", qt)), qpk),
                                   ((("v", kt_b)), vpk)):
                    s = qkvT[src_t][par * 64:(par + 1) * 64, :, rows]\
                        .rearrange("d ge (j b) -> d j ge b", b=8)
                    o = dst[0:64, :]\
                        .rearrange("d (j ge pp b) -> d j ge pp b",
                                   j=16, ge=8, pp=2)[:, :, :, par, :]
                    nc.vector.tensor_copy(out=o, in_=s)
            return kpk, qpk, vpk

        def attn_compute(u, bl, packs, mu_h, vv_h):
            """Attention math for one b-tile from its packed tiles."""
            kpk, qpk, vpk = packs
            # scores + exp, 4 j-groups per PSUM bank
            eT = apool.tile([128, 2048], BF16, tag="eT")
            for sb in range(4):
                sp = psS.tile([128, 512], F32, tag="sp")
                for q in range(4):
                    j = sb * 4 + q
                    nc.tensor.matmul(
                        sp[:, q * 128:(q + 1) * 128],
                        lhsT=kpk[0:73, j * 128:(j + 1) * 128],
                        rhs=qpk[0:73, j * 128:(j + 1) * 128],
                        start=True, stop=True)
                nc.scalar.activation(eT[:, sb * 512:(sb + 1) * 512], sp,
                                     AF.Exp, scale=0.125)
            # vp = transpose(vpk)
            vp = apool.tile([128, 16 * 64], BF16, tag="vp")
            for jb in range(2):
                vt = psT.tile([128, 8 * 64], BF16, tag="pt")
                for jj in range(8):
                    j = jb * 8 + jj
                    nc.tensor.transpose(
                        vt[:, jj * 64:(jj + 1) * 64],
                        vpk[0:64, j * 128:(j + 1) * 128],
                        sb_id[0:64, 0:64])
                nc.vector.tensor_copy(
                    out=vp[:, jb * 512:(jb + 1) * 512], in_=vt)
            # weighted sums + batched row-sum reciprocals + normalize
            caU = apool.tile([128, 1024], BF16, tag="caU", bufs=1)
            for jb in range(2):
                cua = psCU.tile([128, 512], F32, tag="cu")
                cus = psS.tile([128, 8], F32, tag="sp")
                for jj in range(8):
                    j = jb * 8 + jj
                    nc.tensor.matmul(cua[:, jj * 64:(jj + 1) * 64],
                                     lhsT=eT[:, j * 128:(j + 1) * 128],
                                     rhs=vp[:, j * 64:(j + 1) * 64],
                                     start=True, stop=True)
                    nc.tensor.matmul(cus[:, jj:jj + 1],
                                     lhsT=eT[:, j * 128:(j + 1) * 128],
                                     rhs=ones_col,
                                     start=True, stop=True)
                rcz = stp.tile([128, 8], F32, tag="rcz")
                nc.vector.reciprocal(rcz, cus)
                for jj in range(8):
                    j = jb * 8 + jj
                    nc.vector.tensor_scalar(
                        out=caU[:, j * 64:(j + 1) * 64],
                        in0=cua[:, jj * 64:(jj + 1) * 64],
                        scalar1=rcz[:, jj:jj + 1], scalar2=None,
                        op0=AL.mult)
            # transpose caU -> caT_all[:, u*8:(u+1)*8, :]
            for jb in range(2):
                ct = psT.tile([64, 8, 128], BF16, tag="pt")
                for jj in range(8):
                    j = jb * 8 + jj
                    nc.tensor.transpose(ct[0:64, jj, :],
                                        caU[:, j * 64:(j + 1) * 64], sb_id)
                for par in range(2):
                    s = ct[0:64, :, :].rearrange(
                        "d j (hp pp b) -> d hp pp j b", pp=2, b=8)[:, :, par]
                    o = caT_all[par * 64:(par + 1) * 64,
                                u * 8:(u + 1) * 8,
                                jb * 64:(jb + 1) * 64]\
                        .rearrange("d hp (j b) -> d hp j b", b=8)
                    nc.scalar.copy(out=o, in_=s)
            # LN stats (no sqrt here -- batched later)
            sq = apool.tile([128, H // 2, 128], BF16, tag="sq", bufs=1)
            cslice = caT_all[:, u * 8:(u + 1) * 8, :]
            nc.vector.tensor_tensor(out=sq, in0=cslice, in1=cslice,
                                    op=AL.mult)
            mrow = psS.tile([1, 128], F32, tag="sp")
            srow = psS.tile([1, 128], F32, tag="sp")
            for hp in range(H // 2):
                nc.tensor.matmul(mrow, lhsT=ones_col,
                                 rhs=caT_all[:, u * 8 + hp, :],
                                 start=(hp == 0), stop=(hp == 7))
                nc.tensor.matmul(srow, lhsT=ones_col, rhs=sq[:, hp, :],
                                 start=(hp == 0), stop=(hp == 7))
            mus = mu_h[:, bl * 128:(bl + 1) * 128]
            nc.vector.tensor_scalar(
                out=mus, in0=mrow,
                scalar1=1.0 / HID, scalar2=None, op0=AL.mult)
            s2 = stp.tile([1, 128], F32, tag="s2", bufs=2)
            nc.vector.tensor_scalar(out=s2, in0=srow, scalar1=1.0 / HID,
                                    scalar2=EPS, op0=AL.mult, op1=AL.add)
            mu2 = stp.tile([1, 128], F32, tag="mu2", bufs=2)
            nc.vector.tensor_tensor(out=mu2, in0=mus, in1=mus, op=AL.mult)
            nc.vector.tensor_tensor(out=vv_h[:, bl * 128:(bl + 1) * 128],
                                    in0=s2, in1=mu2, op=AL.subtract)

        def proj_half(u0, chunk, t, mu_h, vv_h, extra=()):
            """Projection + residual for units u0..u0+3 (branch t).
            extra: up to 16 thunks interleaved across the 4 n-chunks."""
            # sd = sqrt(vv) as a bf16 row; 1/sd computed later across 128
            # partitions (post-transpose) -- a [1,512] reciprocal runs on a
            # single DVE lane and costs ~3.3us.
            rbsd = stp.tile([1, 512], BF16, tag="rb", bufs=1)
            nc.scalar.activation(rbsd, vv_h, AF.Sqrt)

            def rcol_cols():
                rcp = psT.tile([128, 8], BF16, tag="pt")
                for bl in range(CG // 128):
                    nc.tensor.transpose(rcp[:, 2 * bl:2 * bl + 1],
                                        rbsd[:, bl * 128:(bl + 1) * 128],
                                        sb_id[0:1, 0:1])
                nc.vector.reciprocal(
                    rcol[:, u0:u0 + 4],
                    rcp.rearrange("p (f two) -> p f two", two=2)[:, :, 0])
            def px_mms(bl, nch, wg):
                u = u0 + bl
                cslice = caT_all[:, u * 8:(u + 1) * 8, :]
                pxp = psQ if bl % 2 == 0 else psCU
                px = pxp.tile([128, 512], F32,
                              tag="px" if bl % 2 == 0 else "cu")
                for ct in range(CT):
                    nc.tensor.matmul(px, lhsT=cslice[:, ct, :],
                                     rhs=wg[:, ct, :],
                                     start=(ct == 0), stop=False)
                nc.tensor.matmul(
                    px, lhsT=mu_h[:, bl * 128:(bl + 1) * 128],
                    rhs=sb_un[t][:, nch * 512:(nch + 1) * 512],
                    start=False, stop=True)
                return px

            def px_finish(bl, nch, px):
                u = u0 + bl
                rows = slice(chunk * CG + bl * 128,
                             chunk * CG + (bl + 1) * 128)
                nc.scalar.activation(px, px, AF.Copy,
                                     scale=rcol[:, u:u + 1])
                xres = tmpC.tile([128, 512], BF16, tag="xres")
                nc.sync.dma_start(
                    xres, xv[t][rows, nch * 512:(nch + 1) * 512])
                ot = tmpC.tile([128, 512], F32, tag="ot")
                nc.vector.tensor_tensor(out=ot, in0=px, in1=xres,
                                        op=AL.add)
                nc.sync.dma_start(
                    out[t][rows, nch * 512:(nch + 1) * 512], ot)

            for nch in range(NCH_P):
                for fn in extra[nch * 4:(nch + 1) * 4]:
                    fn()
                wg = wgp.tile([128, CT, 512], BF16, tag="wg")
                nc.sync.dma_start(
                    wg, Wg[t][:, nch * 512:(nch + 1) * 512]
                    .rearrange("(ct p) n -> p ct n", p=128))
                if nch == 0:
                    # all four px groups first (they don't need rcol), the
                    # rcol chain in their shadow, then the scales
                    pxs = [px_mms(bl, 0, wg) for bl in range(CG // 128)]
                    rcol_cols()
                    for bl in range(CG // 128):
                        px_finish(bl, 0, pxs[bl])
                else:
                    for bl in range(CG // 128):
                        px_finish(bl, nch, px_mms(bl, nch, wg))

        def xT_load(chunk, t):
            nc.sync.dma_start(
                xT[t],
                xT_d[t][:, chunk * CG:(chunk + 1) * CG]
                .rearrange("(kt p) r -> p kt r", p=128))

        # ================= main schedule =================
        # Flat 4-half software pipeline over halves i = chunk*2 + half:
        #   qkv(0) | attn(0) x qkv(1) | proj(0) x xT(chunk2) |
        #   attn(1) x qkv(2) | proj(1) | attn(2) x qkv(3) | proj(2) |
        #   attn(3) | proj(3)
        # so the PE always has dense matmul work while attention's
        # DVE/ACT latency chains run underneath.
        def half_params(i):
            chunk, half = divmod(i, 2)
            qt = "c" if half == 0 else "m"
            kt_b = "m" if half == 0 else "c"
            return chunk, half, qt, kt_b

        def qkv_pairs(i):
            _, _, qt, kt_b = half_params(i)
            return [(role, tt, nt)
                    for role, tt in (("q", qt), ("k", kt_b), ("v", kt_b))
                    for nt in range(NT)]

        for t in ("c", "m"):
            xT_load(0, t)
        for pr in qkv_pairs(0):
            qkv_group(*pr)
        for i in range(4):
            chunk, half, qt, kt_b = half_params(i)
            u0 = i * 4
            mu_h = stp.tile([1, 512], BF16, tag="mu_h", bufs=2)
            vv_h = stp.tile([1, 512], F32, tag="vv_h", bufs=2)
            nxt = qkv_pairs(i + 1) if i < 3 else []
            # emit packs one filler-window ahead of their compute so the
            # tail unit's inputs are ready when the PE reaches it
            packs = {0: attn_packs(0, qt, kt_b)}
            for bl in range(CG // 128):
                for pr in nxt[bl * 6:(bl + 1) * 6]:
                    qkv_group(*pr)
                if bl + 1 < CG // 128:
                    packs[bl + 1] = attn_packs(bl + 1, qt, kt_b)
                attn_compute(u0 + bl, bl, packs.pop(bl), mu_h, vv_h)
            extra = []
            if i == 0:
                extra = [(lambda tt=t2: xT_load(1, tt))
                         for t2 in ("c", "m")]
            proj_half(u0, chunk, qt, mu_h, vv_h, extra=extra)
    return nc


_NC = {}


def _get_nc():
    if "nc" not in _NC:
        nc = build_nc()
        if not nc.is_finalized():
            nc.finalize()
        _NC["nc"] = nc
    return _NC["nc"]


def _host_prep(inputs):
    f32 = np.float32
    bf = ml_dtypes.bfloat16
    g = {k: np.asarray(v) for k, v in inputs.items()}
    # permutation: device caT row c_dev (hp*128 + p) <-> ref column d*16+h
    cdev = np.arange(HID)
    hp_t, p_t = cdev // 128, cdev % 128
    h_t = 2 * hp_t + (p_t // 64)
    d_t = p_t % 64
    pr = d_t * H + h_t                   # ref row for each device row
    consts = {}
    for t, (Wp, bp, g1, be1) in (
            ("c", ("W_cproj", "b_cproj", "g1", "be1")),
            ("m", ("W_mproj", "b_mproj", "g2", "be2"))):
        W = np.asarray(g[Wp], f32)[pr, :]          # [HID, CD] permuted
        g1d = np.asarray(g[g1], f32)[pr]
        be1d = np.asarray(g[be1], f32)[pr]
        consts[f"Wg_{t}"] = np.ascontiguousarray(
            (g1d[:, None] * W)).astype(bf)
        consts[f"un_{t}"] = (-(g1d[:, None] * W).sum(0)).reshape(1, CD)
        consts[f"v_{t}"] = (be1d @ W + np.asarray(g[bp], f32)).reshape(1, CD)
    consts["un_all"] = np.concatenate(
        [consts.pop("un_c"), consts.pop("un_m")], 1).astype(bf)
    consts["Wq_c"] = np.asarray(g["W_cqkv"], f32).astype(bf)
    consts["Wq_m"] = np.asarray(g["W_mqkv"], f32).astype(bf)
    consts["bqT_c"] = np.ascontiguousarray(
        np.asarray(g["b_cqkv"], f32).reshape(3 * NT, 128).T)
    consts["bqT_m"] = np.ascontiguousarray(
        np.asarray(g["b_mqkv"], f32).reshape(3 * NT, 128).T)
    # mask extension rows: sum_i kext[i,(g,b)]*qext[i,(h,b')] = MS*(b==b') - MS
    col_b = np.tile(np.arange(128) % 8, 16)        # b index per packed column
    kext = np.zeros((9, 2048), f32)
    qext = np.zeros((9, 2048), f32)
    for i in range(8):
        kext[i] = np.where(col_b == i, MS, 0.0)
        qext[i] = np.where(col_b == i, 1.0, 0.0)
    kext[8] = -MS
    qext[8] = 1.0
    consts["kext"] = kext.astype(bf)
    consts["qext"] = qext.astype(bf)
    consts["identb"] = np.eye(128).astype(bf)
    consts["ones_col"] = np.ones((128, 1)).astype(bf)
    consts["onesr"] = np.ones((1, 128)).astype(bf)
    return g, consts


def kernel(**inputs):
    g, consts = _host_prep(inputs)
    xc = np.ascontiguousarray(np.asarray(g["cnn_out"], np.float32))
    xm = np.ascontiguousarray(np.asarray(g["mlp_out"], np.float32))
    nc = _get_nc()
    v_c = consts.pop("v_c").astype(np.float32)
    v_m = consts.pop("v_m").astype(np.float32)
    bf = ml_dtypes.bfloat16
    xvc = (xc + v_c).astype(bf)
    xvm = (xm + v_m).astype(bf)
    xcb = xc.astype(bf)
    xmb = xm.astype(bf)
    in_maps = []
    for i in range(NCORES):
        m = dict(consts)
        m["xT_c"] = np.ascontiguousarray(xcb[i * BS:(i + 1) * BS].T)
        m["xT_m"] = np.ascontiguousarray(xmb[i * BS:(i + 1) * BS].T)
        m["xv_c"] = xvc[i * BS:(i + 1) * BS]
        m["xv_m"] = xvm[i * BS:(i + 1) * BS]
        in_maps.append(m)
    res = run_bass_kernel_spmd(nc, in_maps, list(range(NCORES))).results
    out_c = np.concatenate([np.asarray(res[i]["out_c"]) for i in range(NCORES)], 0)
    out_m = np.concatenate([np.asarray(res[i]["out_m"]) for i in range(NCORES)], 0)
    return (out_c.astype(np.float32), out_m.astype(np.float32))


# revision 28
# speedup vs baseline: 1.0975x; 1.0847x over previous
"""MultiHeadCrossAttentionFusion kernel for TRN2 (8 NeuronCores, data-parallel over batch).

Per-core design (batch shard BS=1024, processed in 2 chunks of 512 rows):
  - QKV matmuls computed directly in TRANSPOSED layout (weights stationary,
    xT streaming) so attention reads q/k/v with the head dim on partitions.
  - Attention packs 8 samples x 16 heads on partitions; the block-diagonal
    softmax mask rides the score matmul as 9 extra contraction rows
    (mask = 800*delta_bb' - 800 expressed as rank-9 outer products).
  - Softmax denominator via a 65th ones-column appended to V.
  - LN stats via ones-matmuls; all sqrt ops batched (no exp/sqrt ACT-table
    thrash); 1/sd applied to caT with a broadcast-AP multiply; mu*un + v
    folded into the projection as a single K=2 matmul.
  - Projection from caT (stationary) streaming LN-folded weights; residual
    added from a fresh x DMA.
"""
import sys
sys.path.insert(0, "/opt/trn_rl_repo")
import numpy as np
import ml_dtypes
from contextlib import ExitStack

import concourse.bass as bass
from concourse import bacc as _bacc
import concourse.mybir as mybir
from concourse.tile import TileContext
from concourse.bass_utils import run_bass_kernel_spmd

B, CD, HID, H, D = 8192, 2048, 1024, 16, 64
NCORES = 8
BS = B // NCORES          # 1024 rows per core
CG = 512                  # chunk rows
NCHUNK = BS // CG         # 2
KT = CD // 128            # 16 k-tiles for qkv matmul
NT = HID // 128           # 8 n-tiles per role (q/k/v)
CT = HID // 128           # 8 c-tiles for proj contraction
NCH_P = CD // 512         # 4 n-chunks of proj
EPS = 1e-5
MS = 800.0                # mask magnitude (scaled by 1/8 in exp -> -100)
F32 = mybir.dt.float32
BF16 = mybir.dt.bfloat16
AL = mybir.AluOpType
AF = mybir.ActivationFunctionType


def _bc_ap(row_ap, p, reps, n):
    """Broadcast a [1, n] row AP to [p, reps, n] via zero strides."""
    return bass.AP(tensor=row_ap.tensor, offset=row_ap.offset,
                   ap=[[0, p], [0, reps], list(row_ap.ap)[-1][:]])


def build_nc(linearize=False):
    nc = _bacc.Bacc()
    dp = nc.declare_dram_parameter
    xT_d = {"c": dp("xT_c", [CD, BS], BF16, isOutput=False),
            "m": dp("xT_m", [CD, BS], BF16, isOutput=False)}
    Wq = {"c": dp("Wq_c", [CD, 3 * HID], BF16, isOutput=False),
          "m": dp("Wq_m", [CD, 3 * HID], BF16, isOutput=False)}
    bqT = {"c": dp("bqT_c", [128, 3 * NT], F32, isOutput=False),
           "m": dp("bqT_m", [128, 3 * NT], F32, isOutput=False)}
    Wg = {"c": dp("Wg_c", [HID, CD], BF16, isOutput=False),
          "m": dp("Wg_m", [HID, CD], BF16, isOutput=False)}
    xv = {"c": dp("xv_c", [BS, CD], BF16, isOutput=False),
          "m": dp("xv_m", [BS, CD], BF16, isOutput=False)}
    un_d = dp("un_all", [1, 2 * CD], BF16, isOutput=False)
    kext_d = dp("kext", [9, 2048], BF16, isOutput=False)
    qext_d = dp("qext", [9, 2048], BF16, isOutput=False)
    identb = dp("identb", [128, 128], BF16, isOutput=False)
    ones_col_d = dp("ones_col", [128, 1], BF16, isOutput=False)
    onesr_d = dp("onesr", [1, 128], BF16, isOutput=False)
    out = {"c": dp("out_c", [BS, CD], F32, isOutput=True),
           "m": dp("out_m", [BS, CD], F32, isOutput=True)}

    with TileContext(nc, linearize=linearize) as tc, ExitStack() as ctx:
        consts = ctx.enter_context(tc.tile_pool(name="consts", bufs=1))
        keep = ctx.enter_context(tc.tile_pool(name="keep", bufs=1))
        psQ = ctx.enter_context(tc.tile_pool(name="psQ", bufs=2, space="PSUM"))
        psT = ctx.enter_context(tc.tile_pool(name="psT", bufs=2, space="PSUM"))
        psS = ctx.enter_context(tc.tile_pool(name="psS", bufs=2, space="PSUM"))
        psCU = ctx.enter_context(tc.tile_pool(name="psCU", bufs=2, space="PSUM"))
        wst_p = ctx.enter_context(tc.tile_pool(name="wstp", bufs=2))
        apool = ctx.enter_context(tc.tile_pool(name="apool", bufs=2))
        stp = ctx.enter_context(tc.tile_pool(name="stp", bufs=4))
        wgp = ctx.enter_context(tc.tile_pool(name="wgp", bufs=2))
        tmpC = ctx.enter_context(tc.tile_pool(name="tmpC", bufs=2))

        # ---- constants
        sb_id = consts.tile([128, 128], BF16)
        nc.sync.dma_start(sb_id, identb[:, :])
        ones_col = consts.tile([128, 1], BF16)
        nc.sync.dma_start(ones_col, ones_col_d[:, :])
        _ = onesr_d  # unused (kept as a declared param for layout stability)
        sb_un_all = consts.tile([1, 2 * CD], BF16, tag="un")
        nc.sync.dma_start(sb_un_all, un_d[:, :])
        sb_un = {"c": sb_un_all[:, 0:CD], "m": sb_un_all[:, CD:2 * CD]}
        sb_bqT = {}
        for t in ("c", "m"):
            sb_bqT[t] = consts.tile([128, 3 * NT], F32, name=f"bqT_{t}",
                                    tag=f"bqT_{t}")
            nc.sync.dma_start(sb_bqT[t], bqT[t][:, :])

        # ---- persistent cross-chunk tiles
        caT_all = keep.tile([128, 16 * (H // 2), 128], BF16, tag="caT_all")
        rcol = keep.tile([128, 16], F32, tag="rcol")

        # per-chunk qkv-transposed + xT tiles (bufs=1 -> reused across chunks)
        xT = {t: keep.tile([128, KT, CG], BF16, name=f"xT_{t}", tag=f"xT_{t}")
              for t in ("c", "m")}
        qkvT = {}
        for t in ("c", "m"):
            for role in ("q", "k", "v"):
                qkvT[(role, t)] = keep.tile(
                    [128, NT, CG], BF16, name=f"{role}T_{t}",
                    tag=f"{role}T_{t}")

        def qkv_group(role, t, nt):
            """One n-tile of QKV for role/branch on the current chunk rows."""
            roff = {"q": 0, "k": NT, "v": 2 * NT}[role]
            px = psQ.tile([128, CG], F32, tag="px")
            for kh in range(2):
                wst = wst_p.tile([128, KT // 2, 128], BF16, tag="wst",
                                 bufs=5)
                nc.sync.dma_start(
                    wst,
                    Wq[t][kh * 1024:(kh + 1) * 1024,
                          (roff + nt) * 128:(roff + nt + 1) * 128]
                    .rearrange("(kt p) n -> p kt n", p=128))
                for kk in range(KT // 2):
                    kt = kh * 8 + kk
                    nc.tensor.matmul(px, lhsT=wst[:, kk, :],
                                     rhs=xT[t][:, kt, :],
                                     start=(kt == 0),
                                     stop=(kt == KT - 1))
            nc.scalar.activation(
                qkvT[(role, t)][:, nt, :], px, AF.Identity,
                bias=sb_bqT[t][:, roff + nt:roff + nt + 1])

        def attn_packs(bl, qt, kt_b):
            """Build packed K/Q/V tiles for one b-tile (DVE + tiny DMAs)."""
            rows = slice(bl * 128, (bl + 1) * 128)
            kpk = apool.tile([128, 2048], BF16, tag="kpk")
            qpk = apool.tile([128, 2048], BF16, tag="qpk")
            vpk = apool.tile([128, 2048], BF16, tag="vpk")
            nc.sync.dma_start(kpk[64:73, :], kext_d[:, :])
            nc.sync.dma_start(qpk[64:73, :], qext_d[:, :])
            for par in range(2):
                for src_t, dst in (((("k", kt_b)), kpk), ((("q", qt)), qpk),
                                   ((("v", kt_b)), vpk)):
                    s = qkvT[src_t][par * 64:(par + 1) * 64, :, rows]\
                        .rearrange("d ge (j b) -> d j ge b", b=8)
                    o = dst[0:64, :]\
                        .rearrange("d (j ge pp b) -> d j ge pp b",
                                   j=16, ge=8, pp=2)[:, :, :, par, :]
                    nc.vector.tensor_copy(out=o, in_=s)
            return kpk, qpk, vpk

        def attn_compute(u, bl, packs, mu_h, vv_h):
            """Attention math for one b-tile from its packed tiles."""
            kpk, qpk, vpk = packs
            # scores + exp, 4 j-groups per PSUM bank
            eT = apool.tile([128, 2048], BF16, tag="eT")
            for sb in range(4):
                sp = psS.tile([128, 512], F32, tag="sp")
                for q in range(4):
                    j = sb * 4 + q
                    nc.tensor.matmul(
                        sp[:, q * 128:(q + 1) * 128],
                        lhsT=kpk[0:73, j * 128:(j + 1) * 128],
                        rhs=qpk[0:73, j * 128:(j + 1) * 128],
                        start=True, stop=True)
                nc.scalar.activation(eT[:, sb * 512:(sb + 1) * 512], sp,
                                     AF.Exp, scale=0.125)
            # vp = transpose(vpk)
            vp = apool.tile([128, 16 * 64], BF16, tag="vp")
            for jb in range(2):
                vt = psT.tile([128, 8 * 64], BF16, tag="pt")
                for jj in range(8):
                    j = jb * 8 + jj
                    nc.tensor.transpose(
                        vt[:, jj * 64:(jj + 1) * 64],
                        vpk[0:64, j * 128:(j + 1) * 128],
                        sb_id[0:64, 0:64])
                nc.vector.tensor_copy(
                    out=vp[:, jb * 512:(jb + 1) * 512], in_=vt)
            # weighted sums + batched row-sum reciprocals + normalize
            caU = apool.tile([128, 1024], BF16, tag="caU", bufs=1)
            for jb in range(2):
                cua = psCU.tile([128, 512], F32, tag="cu")
                cus = psS.tile([128, 8], F32, tag="sp")
                for jj in range(8):
                    j = jb * 8 + jj
                    nc.tensor.matmul(cua[:, jj * 64:(jj + 1) * 64],
                                     lhsT=eT[:, j * 128:(j + 1) * 128],
                                     rhs=vp[:, j * 64:(j + 1) * 64],
                                     start=True, stop=True)
                    nc.tensor.matmul(cus[:, jj:jj + 1],
                                     lhsT=eT[:, j * 128:(j + 1) * 128],
                                     rhs=ones_col,
                                     start=True, stop=True)
                rcz = stp.tile([128, 8], F32, tag="rcz")
                nc.vector.reciprocal(rcz, cus)
                for jj in range(8):
                    j = jb * 8 + jj
                    nc.vector.tensor_scalar(
                        out=caU[:, j * 64:(j + 1) * 64],
                        in0=cua[:, jj * 64:(jj + 1) * 64],
                        scalar1=rcz[:, jj:jj + 1], scalar2=None,
                        op0=AL.mult)
            # transpose caU -> caT_all[:, u*8:(u+1)*8, :]
            for jb in range(2):
                ct = psT.tile([64, 8, 128], BF16, tag="pt")
                for jj in range(8):
                    j = jb * 8 + jj
                    nc.tensor.transpose(ct[0:64, jj, :],
                                        caU[:, j * 64:(j + 1) * 64], sb_id)
                for par in range(2):
                    s = ct[0:64, :, :].rearrange(
                        "d j (hp pp b) -> d hp pp j b", pp=2, b=8)[:, :, par]
                    o = caT_all[par * 64:(par + 1) * 64,
                                u * 8:(u + 1) * 8,
                                jb * 64:(jb + 1) * 64]\
                        .rearrange("d hp (j b) -> d hp j b", b=8)
                    nc.scalar.copy(out=o, in_=s)
            # LN stats (no sqrt here -- batched later)
            sq = apool.tile([128, H // 2, 128], BF16, tag="sq", bufs=1)
            cslice = caT_all[:, u * 8:(u + 1) * 8, :]
            nc.vector.tensor_tensor(out=sq, in0=cslice, in1=cslice,
                                    op=AL.mult)
            mrow = psS.tile([1, 128], F32, tag="sp")
            srow = psS.tile([1, 128], F32, tag="sp")
            for hp in range(H // 2):
                nc.tensor.matmul(mrow, lhsT=ones_col,
                                 rhs=caT_all[:, u * 8 + hp, :],
                                 start=(hp == 0), stop=(hp == 7))
                nc.tensor.matmul(srow, lhsT=ones_col, rhs=sq[:, hp, :],
                                 start=(hp == 0), stop=(hp == 7))
            mus = mu_h[:, bl * 128:(bl + 1) * 128]
            nc.vector.tensor_scalar(
                out=mus, in0=mrow,
                scalar1=1.0 / HID, scalar2=None, op0=AL.mult)
            s2 = stp.tile([1, 128], F32, tag="s2", bufs=2)
            nc.vector.tensor_scalar(out=s2, in0=srow, scalar1=1.0 / HID,
                                    scalar2=EPS, op0=AL.mult, op1=AL.add)
            mu2 = stp.tile([1, 128], F32, tag="mu2", bufs=2)
            nc.vector.tensor_tensor(out=mu2, in0=mus, in1=mus, op=AL.mult)
            nc.vector.tensor_tensor(out=vv_h[:, bl * 128:(bl + 1) * 128],
                                    in0=s2, in1=mu2, op=AL.subtract)

        def proj_half(u0, chunk, t, mu_h, vv_h, extra=(), defer=False):
            """Projection + residual for units u0..u0+3 (branch t).
            extra: up to 16 thunks interleaved across the 4 n-chunks.
            defer=True returns the 4 n-chunk blocks as thunks instead of
            emitting them (used to fill the last attention window)."""
            # sd = sqrt(vv) as a bf16 row; 1/sd computed later across 128
            # partitions (post-transpose) -- a [1,512] reciprocal runs on a
            # single DVE lane and costs ~3.3us.
            rbsd = stp.tile([1, 512], BF16, tag="rb", bufs=2)

            def rcol_cols():
                rcp = psT.tile([128, 8], BF16, tag="pt")
                for bl in range(CG // 128):
                    nc.tensor.transpose(rcp[:, 2 * bl:2 * bl + 1],
                                        rbsd[:, bl * 128:(bl + 1) * 128],
                                        sb_id[0:1, 0:1])
                nc.vector.reciprocal(
                    rcol[:, u0:u0 + 4],
                    rcp.rearrange("p (f two) -> p f two", two=2)[:, :, 0])
            def px_mms(bl, nch, wg):
                u = u0 + bl
                cslice = caT_all[:, u * 8:(u + 1) * 8, :]
                pxp = psQ if bl % 2 == 0 else psCU
                px = pxp.tile([128, 512], F32,
                              tag="px" if bl % 2 == 0 else "cu")
                for ct in range(CT):
                    nc.tensor.matmul(px, lhsT=cslice[:, ct, :],
                                     rhs=wg[:, ct, :],
                                     start=(ct == 0), stop=False)
                nc.tensor.matmul(
                    px, lhsT=mu_h[:, bl * 128:(bl + 1) * 128],
                    rhs=sb_un[t][:, nch * 512:(nch + 1) * 512],
                    start=False, stop=True)
                return px

            def px_finish(bl, nch, px):
                u = u0 + bl
                rows = slice(chunk * CG + bl * 128,
                             chunk * CG + (bl + 1) * 128)
                nc.scalar.activation(px, px, AF.Copy,
                                     scale=rcol[:, u:u + 1])
                xres = tmpC.tile([128, 512], BF16, tag="xres")
                nc.sync.dma_start(
                    xres, xv[t][rows, nch * 512:(nch + 1) * 512])
                ot = tmpC.tile([128, 512], F32, tag="ot")
                nc.vector.tensor_tensor(out=ot, in0=px, in1=xres,
                                        op=AL.add)
                nc.sync.dma_start(
                    out[t][rows, nch * 512:(nch + 1) * 512], ot)

            def nch_block(nch):
                for fn in extra[nch * 4:(nch + 1) * 4]:
                    fn()
                if nch == 0:
                    nc.scalar.activation(rbsd, vv_h, AF.Sqrt)
                wg = wgp.tile([128, CT, 512], BF16, tag="wg")
                nc.sync.dma_start(
                    wg, Wg[t][:, nch * 512:(nch + 1) * 512]
                    .rearrange("(ct p) n -> p ct n", p=128))
                if nch == 0:
                    # all four px groups first (they don't need rcol), the
                    # rcol chain in their shadow, then the scales
                    pxs = [px_mms(bl, 0, wg) for bl in range(CG // 128)]
                    rcol_cols()
                    for bl in range(CG // 128):
                        px_finish(bl, 0, pxs[bl])
                else:
                    for bl in range(CG // 128):
                        px_finish(bl, nch, px_mms(bl, nch, wg))

            blocks = [lambda n=nch: nch_block(n) for nch in range(NCH_P)]
            if defer:
                return blocks
            for b in blocks:
                b()

        def xT_load(chunk, t):
            nc.sync.dma_start(
                xT[t],
                xT_d[t][:, chunk * CG:(chunk + 1) * CG]
                .rearrange("(kt p) r -> p kt r", p=128))

        # ================= main schedule =================
        # Flat 4-half software pipeline over halves i = chunk*2 + half:
        #   qkv(0) | attn(0) x qkv(1) | proj(0) x xT(chunk2) |
        #   attn(1) x qkv(2) | proj(1) | attn(2) x qkv(3) | proj(2) |
        #   attn(3) | proj(3)
        # so the PE always has dense matmul work while attention's
        # DVE/ACT latency chains run underneath.
        def half_params(i):
            chunk, half = divmod(i, 2)
            qt = "c" if half == 0 else "m"
            kt_b = "m" if half == 0 else "c"
            return chunk, half, qt, kt_b

        def qkv_pairs(i):
            _, _, qt, kt_b = half_params(i)
            return [(role, tt, nt)
                    for role, tt in (("q", qt), ("k", kt_b), ("v", kt_b))
                    for nt in range(NT)]

        for t in ("c", "m"):
            xT_load(0, t)
        for pr in qkv_pairs(0):
            qkv_group(*pr)
        deferred = None
        for i in range(4):
            chunk, half, qt, kt_b = half_params(i)
            u0 = i * 4
            mu_h = stp.tile([1, 512], BF16, tag="mu_h", bufs=2)
            vv_h = stp.tile([1, 512], F32, tag="vv_h", bufs=2)
            nxt = qkv_pairs(i + 1) if i < 3 else []
            # emit packs one filler-window ahead of their compute so the
            # tail unit's inputs are ready when the PE reaches it
            packs = {0: attn_packs(0, qt, kt_b)}
            for bl in range(CG // 128):
                for pr in nxt[bl * 6:(bl + 1) * 6]:
                    qkv_group(*pr)
                if deferred is not None:
                    deferred[bl]()
                if bl + 1 < CG // 128:
                    packs[bl + 1] = attn_packs(bl + 1, qt, kt_b)
                attn_compute(u0 + bl, bl, packs.pop(bl), mu_h, vv_h)
            deferred = None
            extra = []
            if i == 0:
                extra = [(lambda tt=t2: xT_load(1, tt))
                         for t2 in ("c", "m")]
            if i == 2:
                # half 2's projection fills half 3's attention window
                deferred = proj_half(u0, chunk, qt, mu_h, vv_h,
                                     extra=extra, defer=True)
            else:
                proj_half(u0, chunk, qt, mu_h, vv_h, extra=extra)
    return nc


_NC = {}


def _get_nc():
    if "nc" not in _NC:
        nc = build_nc()
        if not nc.is_finalized():
            nc.finalize()
        _NC["nc"] = nc
    return _NC["nc"]


def _host_prep(inputs):
    f32 = np.float32
    bf = ml_dtypes.bfloat16
    g = {k: np.asarray(v) for k, v in inputs.items()}
    # permutation: device caT row c_dev (hp*128 + p) <-> ref column d*16+h
    cdev = np.arange(HID)
    hp_t, p_t = cdev // 128, cdev % 128
    h_t = 2 * hp_t + (p_t // 64)
    d_t = p_t % 64
    pr = d_t * H + h_t                   # ref row for each device row
    consts = {}
    for t, (Wp, bp, g1, be1) in (
            ("c", ("W_cproj", "b_cproj", "g1", "be1")),
            ("m", ("W_mproj", "b_mproj", "g2", "be2"))):
        W = np.asarray(g[Wp], f32)[pr, :]          # [HID, CD] permuted
        g1d = np.asarray(g[g1], f32)[pr]
        be1d = np.asarray(g[be1], f32)[pr]
        consts[f"Wg_{t}"] = np.ascontiguousarray(
            (g1d[:, None] * W)).astype(bf)
        consts[f"un_{t}"] = (-(g1d[:, None] * W).sum(0)).reshape(1, CD)
        consts[f"v_{t}"] = (be1d @ W + np.asarray(g[bp], f32)).reshape(1, CD)
    consts["un_all"] = np.concatenate(
        [consts.pop("un_c"), consts.pop("un_m")], 1).astype(bf)
    consts["Wq_c"] = np.asarray(g["W_cqkv"], f32).astype(bf)
    consts["Wq_m"] = np.asarray(g["W_mqkv"], f32).astype(bf)
    consts["bqT_c"] = np.ascontiguousarray(
        np.asarray(g["b_cqkv"], f32).reshape(3 * NT, 128).T)
    consts["bqT_m"] = np.ascontiguousarray(
        np.asarray(g["b_mqkv"], f32).reshape(3 * NT, 128).T)
    # mask extension rows: sum_i kext[i,(g,b)]*qext[i,(h,b')] = MS*(b==b') - MS
    col_b = np.tile(np.arange(128) % 8, 16)        # b index per packed column
    kext = np.zeros((9, 2048), f32)
    qext = np.zeros((9, 2048), f32)
    for i in range(8):
        kext[i] = np.where(col_b == i, MS, 0.0)
        qext[i] = np.where(col_b == i, 1.0, 0.0)
    kext[8] = -MS
    qext[8] = 1.0
    consts["kext"] = kext.astype(bf)
    consts["qext"] = qext.astype(bf)
    consts["identb"] = np.eye(128).astype(bf)
    consts["ones_col"] = np.ones((128, 1)).astype(bf)
    consts["onesr"] = np.ones((1, 128)).astype(bf)
    return g, consts


def kernel(**inputs):
    g, consts = _host_prep(inputs)
    xc = np.ascontiguousarray(np.asarray(g["cnn_out"], np.float32))
    xm = np.ascontiguousarray(np.asarray(g["mlp_out"], np.float32))
    nc = _get_nc()
    v_c = consts.pop("v_c").astype(np.float32)
    v_m = consts.pop("v_m").astype(np.float32)
    bf = ml_dtypes.bfloat16
    xvc = (xc + v_c).astype(bf)
    xvm = (xm + v_m).astype(bf)
    xcb = xc.astype(bf)
    xmb = xm.astype(bf)
    in_maps = []
    for i in range(NCORES):
        m = dict(consts)
        m["xT_c"] = np.ascontiguousarray(xcb[i * BS:(i + 1) * BS].T)
        m["xT_m"] = np.ascontiguousarray(xmb[i * BS:(i + 1) * BS].T)
        m["xv_c"] = xvc[i * BS:(i + 1) * BS]
        m["xv_m"] = xvm[i * BS:(i + 1) * BS]
        in_maps.append(m)
    res = run_bass_kernel_spmd(nc, in_maps, list(range(NCORES))).results
    out_c = np.concatenate([np.asarray(res[i]["out_c"]) for i in range(NCORES)], 0)
    out_m = np.concatenate([np.asarray(res[i]["out_m"]) for i in range(NCORES)], 0)
    return (out_c.astype(np.float32), out_m.astype(np.float32))
